# revision 1
# baseline (speedup 1.0000x reference)
"""Trainium2 kernel for nn_AdaptiveFeaturePooling (2-level ROI-align + cross-level max).

Math: every box is exactly 14x14 in image coords, so torchvision roi_align
(aligned=True) collapses to a separable stencil per box and pyramid level:

  level l (scale s in {1,2}, grid g = 2s, patch rows N = 14s+1):
    y0 = floor(s*y1), fy = frac(s*y1)   (same for x)
    out[p,u] = sum_{t,s'} wy[t] * wx[s'] * F[y0+g*p+t, x0+g*u+s']
    wy = [(1-fy), 1, ..., 1, fy] / g    (g+1 taps)

Sampling positions never go out of bounds or hit the H-1 clamp for the
reference's box distribution (top-left in [0, 114)).

Device strategy (8 cores, 64 boxes each):
  - host passes channels-last fp16 feature maps, per-core flat gather
    offsets (int32), per-core block-diagonal y-stage weights (fp16), and
    per-core x-fraction vectors. One SPMD NEFF; per-core variation is data.
  - indirect DMA gathers patch rows: partition <- one (y-row, x-window)
    strip of 29*128 (L1) or 15*128 (L0) contiguous fp16 values.
    NOTE: the HW indirect-DMA path requires a 2-D dst AP.
  - y-contraction on TensorE: fp16 lhsT [128,112] block-diagonal, 4 (L1) /
    2 (L0) matmuls accumulate 16 boxes' rows into f32 PSUM [112,(j,c32)]
    per c-quarter; weight-outer loop order to minimize weight reloads.
  - x-contraction: ScalarE evacuates PSUM (cast fp16), VectorE applies the
    boundary taps with per-partition fx scalars via scalar_tensor_tensor.
  - cross-level max on VectorE (f32), DMA out, host reassembles.
"""

import sys

sys.path.insert(0, "/opt/trn_rl_repo")

import numpy as np

import concourse.bass as bass
import concourse.mybir as mybir
import concourse.tile as tile
from concourse import bacc
from concourse.bass_utils import run_bass_kernel_spmd

F32 = mybir.dt.float32
F16 = mybir.dt.float16
I32 = mybir.dt.int32

N_CORES = 8
K_TOTAL = 512
K_CORE = K_TOTAL // N_CORES      # 64 boxes per core
N_BATCH = 4                      # 16-box batches per core
BOX_B = 16                       # boxes per batch

# level geometry
H1 = 256
H0 = 128
C = 128
NJ1 = 29                         # level-1 patch width (j cells)
NJ0 = 15
SLOT1 = 32                       # partition row-slot per box, level 1 (4 boxes)
SLOT0 = 16                       # level 0 (8 boxes)


def build_nc():
    nc = bacc.Bacc()
    f0 = nc.declare_dram_parameter("f0", [2, H0, H0, C], F16, isOutput=False)
    f1 = nc.declare_dram_parameter("f1", [2, H1, H1, C], F16, isOutput=False)
    idx1 = nc.declare_dram_parameter("idx1", [128, 16], I32, isOutput=False)
    idx0 = nc.declare_dram_parameter("idx0", [128, 8], I32, isOutput=False)
    w1 = nc.declare_dram_parameter("w1", [128, 16, 112], F16, isOutput=False)
    w0 = nc.declare_dram_parameter("w0", [128, 8, 112], F16, isOutput=False)
    # fxv columns per batch b: 4b+0=fx1, 4b+1=1-fx1, 4b+2=fx0, 4b+3=1-fx0
    fxv = nc.declare_dram_parameter("fxv", [112, 16], F32, isOutput=False)
    out = nc.declare_dram_parameter("out", [N_BATCH, 112, 7, C], F32, isOutput=True)

    f0_2d = f0[:].rearrange("b h w c -> (b h w) c")
    f1_2d = f1[:].rearrange("b h w c -> (b h w) c")

    with tile.TileContext(nc) as tc:
        with (
            tc.tile_pool(name="const", bufs=1) as cpool,
            tc.tile_pool(name="p1", bufs=12) as p1pool,
            tc.tile_pool(name="p0", bufs=8) as p0pool,
            tc.tile_pool(name="ev", bufs=4) as evpool,
            tc.tile_pool(name="tmp", bufs=6) as tpool,
            tc.tile_pool(name="o", bufs=4) as opool,
            tc.tile_pool(name="ps1", bufs=3, space="PSUM") as ps1pool,
            tc.tile_pool(name="ps0", bufs=2, space="PSUM") as ps0pool,
        ):
            idx0_t = cpool.tile([128, 8], I32)
            nc.sync.dma_start(idx0_t[:], idx0[:])
            idx1_t = cpool.tile([128, 16], I32)
            nc.sync.dma_start(idx1_t[:], idx1[:])
            w1_t = cpool.tile([128, 16, 112], F16)
            nc.sync.dma_start(w1_t[:], w1[:])
            w0_t = cpool.tile([128, 8, 112], F16)
            nc.sync.dma_start(w0_t[:], w0[:])
            fxv_t = cpool.tile([112, 16], F32)
            nc.sync.dma_start(fxv_t[:], fxv[:])

            for b in range(N_BATCH):
                patch1 = []
                for g in range(4):
                    pt = p1pool.tile([128, NJ1, C], F16, tag="p1")
                    nc.gpsimd.indirect_dma_start(
                        out=pt[:].rearrange("p a c -> p (a c)"),
                        out_offset=None,
                        in_=f1_2d,
                        in_offset=bass.IndirectOffsetOnAxis(
                            ap=idx1_t[:, 4 * b + g : 4 * b + g + 1], axis=1
                        ),
                    )
                    patch1.append(pt)
                patch0 = []
                for h in range(2):
                    pt = p0pool.tile([128, NJ0, C], F16, tag="p0")
                    nc.gpsimd.indirect_dma_start(
                        out=pt[:].rearrange("p a c -> p (a c)"),
                        out_offset=None,
                        in_=f0_2d,
                        in_offset=bass.IndirectOffsetOnAxis(
                            ap=idx0_t[:, 2 * b + h : 2 * b + h + 1], axis=1
                        ),
                    )
                    patch0.append(pt)

                o1 = opool.tile([112, 7, C], F32, tag="o1")
                o0 = opool.tile([112, 7, C], F32, tag="o0")
                fx1 = fxv_t[:, 4 * b + 0 : 4 * b + 1]
                fx1m = fxv_t[:, 4 * b + 1 : 4 * b + 2]
                fx0 = fxv_t[:, 4 * b + 2 : 4 * b + 3]
                fx0m = fxv_t[:, 4 * b + 3 : 4 * b + 4]

                for qp in (0, 2):
                    qs = (qp, qp + 1)
                    # ---- level 1 y-contraction (weight-outer to cut reloads)
                    ps1 = {
                        q: ps1pool.tile(
                            [112, NJ1, 32], F32, space="PSUM", tag="ps1", name=f"ps1_{q}"
                        )
                        for q in qs
                    }
                    for g in range(4):
                        for q in qs:
                            cs = slice(32 * q, 32 * q + 32)
                            for j0, j1 in ((0, 16), (16, NJ1)):
                                nc.tensor.matmul(
                                    ps1[q][:, j0:j1, :],
                                    w1_t[:, 4 * b + g, :],
                                    patch1[g][:, j0:j1, cs],
                                    start=(g == 0),
                                    stop=(g == 3),
                                    skip_group_check=True,
                                )
                    # ---- level 0 y-contraction
                    ps0 = {
                        q: ps0pool.tile(
                            [112, NJ0, 32], F32, space="PSUM", tag="ps0", name=f"ps0_{q}"
                        )
                        for q in qs
                    }
                    for h in range(2):
                        for q in qs:
                            cs = slice(32 * q, 32 * q + 32)
                            nc.tensor.matmul(
                                ps0[q][:],
                                w0_t[:, 2 * b + h, :],
                                patch0[h][:, :, cs],
                                start=(h == 0),
                                stop=(h == 1),
                                skip_group_check=True,
                            )
                    # ---- x-contraction + write level outputs
                    for q in qs:
                        cs = slice(32 * q, 32 * q + 32)
                        ev = evpool.tile([112, NJ1, 32], F16, tag="ev1")
                        nc.scalar.copy(ev[:], ps1[q][:])
                        s = [ev[:, t : t + 25 : 4, :] for t in range(5)]
                        a1 = tpool.tile([112, 7, 32], F16, tag="a1")
                        a2 = tpool.tile([112, 7, 32], F16, tag="a2")
                        nc.vector.scalar_tensor_tensor(
                            a1[:], s[4], fx1, s[1],
                            mybir.AluOpType.mult, mybir.AluOpType.add,
                        )
                        nc.vector.scalar_tensor_tensor(
                            a2[:], s[0], fx1m, s[2],
                            mybir.AluOpType.mult, mybir.AluOpType.add,
                        )
                        nc.vector.tensor_add(a1[:], a1[:], a2[:])
                        nc.vector.tensor_add(o1[:, :, cs], a1[:], s[3])

                        ev0 = evpool.tile([112, NJ0, 32], F16, tag="ev0")
                        nc.scalar.copy(ev0[:], ps0[q][:])
                        s0 = [ev0[:, t : t + 13 : 2, :] for t in range(3)]
                        a3 = tpool.tile([112, 7, 32], F16, tag="a3")
                        nc.vector.scalar_tensor_tensor(
                            a3[:], s0[2], fx0, s0[1],
                            mybir.AluOpType.mult, mybir.AluOpType.add,
                        )
                        nc.vector.scalar_tensor_tensor(
                            o0[:, :, cs], s0[0], fx0m, a3[:],
                            mybir.AluOpType.mult, mybir.AluOpType.add,
                        )

                nc.vector.tensor_max(o1[:], o1[:], o0[:])
                nc.sync.dma_start(out[b], o1[:])
    nc.finalize()
    return nc


def _per_core_aux(boxes, bbi, r):
    """Host-side index/weight construction for core r (boxes 64r..64r+63)."""
    idx1 = np.zeros((128, 16), np.int32)
    idx0 = np.zeros((128, 8), np.int32)
    w1 = np.zeros((128, 16, 112), np.float32)
    w0 = np.zeros((128, 8, 112), np.float32)
    fxv = np.zeros((112, 16), np.float32)

    for t in range(K_CORE):
        k = K_CORE * r + t
        b, kb = divmod(t, BOX_B)
        bidx = int(bbi[k])
        x1, y1 = float(boxes[k, 0]), float(boxes[k, 1])
        pp0 = 28 * (kb // 4) + 7 * (kb % 4)          # psum partition base

        # level 1 (scale 2, grid 4)
        sx, sy = 2.0 * x1, 2.0 * y1
        x0i, y0i = int(np.floor(sx)), int(np.floor(sy))
        x0i = max(0, min(x0i, H1 - NJ1))
        y0i = max(0, min(y0i, H1 - NJ1))
        fxs, fys = np.float32(sx - x0i), np.float32(sy - y0i)
        g, m = kb // 4, kb % 4
        rows = SLOT1 * m
        for i in range(SLOT1):
            yi = y0i + min(i, NJ1 - 1)
            idx1[rows + i, 4 * b + g] = ((bidx * H1 + yi) * H1 + x0i) * C
        cy = np.array([1.0 - fys, 1.0, 1.0, 1.0, fys], np.float64) / 16.0
        for p in range(7):
            for tt in range(5):
                w1[rows + 4 * p + tt, 4 * b + g, pp0 + p] += cy[tt]
        fxv[pp0 : pp0 + 7, 4 * b + 0] = fxs
        fxv[pp0 : pp0 + 7, 4 * b + 1] = np.float32(1.0) - fxs

        # level 0 (scale 1, grid 2)
        x0i, y0i = int(np.floor(x1)), int(np.floor(y1))
        x0i = max(0, min(x0i, H0 - NJ0))
        y0i = max(0, min(y0i, H0 - NJ0))
        fxs, fys = np.float32(x1 - x0i), np.float32(y1 - y0i)
        h, m8 = kb // 8, kb % 8
        rows = SLOT0 * m8
        for i in range(SLOT0):
            yi = y0i + min(i, NJ0 - 1)
            idx0[rows + i, 2 * b + h] = ((bidx * H0 + yi) * H0 + x0i) * C
        cy = np.array([1.0 - fys, 1.0, fys], np.float64) / 4.0
        for p in range(7):
            for tt in range(3):
                w0[rows + 2 * p + tt, 2 * b + h, pp0 + p] += cy[tt]
        fxv[pp0 : pp0 + 7, 4 * b + 2] = fxs
        fxv[pp0 : pp0 + 7, 4 * b + 3] = np.float32(1.0) - fxs

    return dict(
        idx1=idx1,
        idx0=idx0,
        w1=w1.astype(np.float16),
        w0=w0.astype(np.float16),
        fxv=fxv,
    )


def make_in_maps(feat0, feat1, boxes, box_batch_idx):
    f0 = np.ascontiguousarray(
        np.transpose(np.asarray(feat0, np.float32), (0, 2, 3, 1))
    ).astype(np.float16)
    f1 = np.ascontiguousarray(
        np.transpose(np.asarray(feat1, np.float32), (0, 2, 3, 1))
    ).astype(np.float16)
    boxes = np.asarray(boxes, np.float32)
    bbi = np.asarray(box_batch_idx, np.int32)
    in_maps = []
    for r in range(N_CORES):
        m = _per_core_aux(boxes, bbi, r)
        m["f0"] = f0
        m["f1"] = f1
        in_maps.append(m)
    return in_maps


def assemble(results):
    """results: 8 dicts with 'out' [4, 112, 7, 128] -> [512, 128, 7, 7]."""
    outs = []
    for r in range(N_CORES):
        a = np.asarray(results[r]["out"])          # [4, 112, 7, 128]
        a = a.reshape(N_BATCH, 4, 4, 7, 7, C)      # [b, g, m, p, u, c]
        a = a.transpose(0, 1, 2, 5, 3, 4)          # [b, g, m, c, p, u]
        outs.append(a.reshape(K_CORE, C, 7, 7))
    return np.concatenate(outs, axis=0)


_NC_CACHE = None


def run(inputs, **spmd_kwargs):
    global _NC_CACHE
    if _NC_CACHE is None:
        _NC_CACHE = build_nc()
    in_maps = make_in_maps(
        inputs["feat0"], inputs["feat1"], inputs["boxes"], inputs["box_batch_idx"]
    )
    res = run_bass_kernel_spmd(
        _NC_CACHE, in_maps, core_ids=list(range(N_CORES)), **spmd_kwargs
    )
    return assemble(res.results), res


def kernel(feat0, feat1, boxes, box_batch_idx):
    out, _ = run(
        dict(feat0=feat0, feat1=feat1, boxes=boxes, box_batch_idx=box_batch_idx)
    )
    return out


if __name__ == "__main__":
    # smoke test against the reference when run inside /root/problem
    import reference

    inputs = {k: np.asarray(v) for k, v in reference.setup_inputs().items()}
    got = kernel(**inputs)
    exp = np.asarray(reference.reference(**inputs))
    num = np.linalg.norm((got - exp).ravel())
    den = np.linalg.norm(exp.ravel())
    print("Relative error:", num / den)

